# revision 1
# baseline (speedup 1.0000x reference)
"""Trainium2 Bass kernel for the attention-like exp/reduce problem.

Math (per batch element b, fully data-parallel across 8 cores):
    M[p, q]      = exp(dec[p] * enc[t, q])            (256x256 per timestep t)
    colsum[t,q]  = sum_p M[p, q]
    rowsum[t,q]  = sum_r exp(dec[q] * enc[t, r])
    out[q]       = sum_t enc[t,q] * colsum[t,q] / rowsum[t,q]

Implementation notes:
  * M is materialized once per core in orientation [i (dec idx, partition), (t, j) free]
    as exp(dec[i] * enc[t, j]): enc rows are broadcast across all 128 partitions by a
    0-stride DMA, then one ACT instruction per (chunk, i-half) applies
    exp(scale * x) with per-partition scale = dec[i].
  * M is stored in bf16 (band entries 0/1 are exact; M's rounding noise averages
    out in the 256-term sums; both colsum and rowsum share the same rounded M,
    keeping the ratio well-conditioned).
  * rowsum[t, q] == per-partition segmented free-axis sums of M: a bf16 pairwise
    fold (tensor_tensor add, fast packed mode) followed by a DVE tensor_reduce with
    a 3D access pattern (innermost axis) writes rowsum^T in [q, t] layout directly.
  * colsum[t, q] == partition-axis sums of M. Done on the tensor engine: stationary
    lhsT is a slice of a "band" matrix (all-ones column), lhsT[i, m] = 1 iff m == t,
    so out[m, :] += (m == t) ? colsum_t : 0 accumulates the whole [t, q] colsum
    matrix into a single PSUM tile across 256 matmuls.
  * combine: reciprocal on DVE, PE transpose of rowsum^T, scalar_tensor_tensor
    reading colsum straight from PSUM, final all-ones-column matmul contracts over
    t, DMA out. Chunk sizes ramp up/down ([4,4,8,8]...[12,4]) to fill and drain the
    ACT/DVE/PE/DMA pipeline quickly.
"""

import sys

sys.path.insert(0, "/opt/trn_rl_repo")

import numpy as np

import concourse.bass as bass
import concourse.bacc as bacc
import concourse.tile as tile
from concourse import mybir
from concourse.bass_utils import run_bass_kernel_spmd

# The agent image's antenv package lacks axon_hooks; if BASS_TRACE is set in the
# environment, run_bass_kernel_spmd would die on the import. Provide a stub that
# reports "no hook" so tracing degrades gracefully instead. (A real hook installed
# earlier, e.g. by a profiling harness, is left untouched.)
try:
    import antenv.axon_hooks  # noqa: F401
except ImportError:
    import types

    import antenv

    _hooks = types.ModuleType("antenv.axon_hooks")
    _hooks.get_axon_ntff_profile_hook = lambda: None
    _hooks.set_axon_ntff_profile_hook = lambda h: None
    sys.modules["antenv.axon_hooks"] = _hooks
    antenv.axon_hooks = _hooks

B, T, D = 8, 128, 256
NCORES = 8
TC = 16  # timesteps per chunk
ACCUM_T = 0  # timesteps per (chunk, half) routed via ACT accum_out instead of DVE
F32 = mybir.dt.float32
F32R = mybir.dt.float32r
BF16 = mybir.dt.bfloat16
EXP = mybir.ActivationFunctionType.Exp


def _band_np():
    import ml_dtypes
    band = np.zeros((128, 257), dtype=ml_dtypes.bfloat16)
    band[:, 128] = 1.0
    return band


def _ident_np():
    return np.eye(128, dtype=np.float32)


def build_nc():
    nc = bacc.Bacc("TRN2")
    dec2 = nc.dram_tensor("dec2", [128, 2], F32, kind="ExternalInput").ap()
    enc = nc.dram_tensor("enc", [T, D], F32, kind="ExternalInput").ap()
    band = nc.dram_tensor("band", [128, 257], BF16, kind="ExternalInput").ap()
    onescol = nc.dram_tensor("onescol", [128, 1], BF16, kind="ExternalInput").ap()
    ident = nc.dram_tensor("ident", [128, 128], F32, kind="ExternalInput").ap()
    out = nc.dram_tensor("out", [1, D], F32, kind="ExternalOutput").ap()

    ramp, tail = [4, 4, 8, 8], [12, 4]
    mid = T - sum(ramp) - sum(tail)
    chunk_sizes = ramp + [TC] * (mid // TC) + ([mid % TC] if mid % TC else []) + tail
    assert sum(chunk_sizes) == T, chunk_sizes
    n_cs_mms = 2 * T  # two i-halves per timestep

    with tile.TileContext(nc) as tc:
        with (
            tc.tile_pool(name="const", bufs=1) as constp,
            tc.tile_pool(name="bc", bufs=4) as bcp,
            tc.tile_pool(name="m", bufs=4) as mp,
            tc.tile_pool(name="fold", bufs=3) as foldp,
            tc.tile_pool(name="cs", bufs=1, space="PSUM") as csp,
            tc.tile_pool(name="tp", bufs=2, space="PSUM") as tpp,
        ):
            warm = constp.tile([128, 1], F32, tag="warm")
            nc.vector.memset(warm[:], 0.0)
            nc.scalar.activation(warm[:], warm[:], EXP)
            dec_sb = constp.tile([128, 2], F32, tag="dec")
            nc.gpsimd.dma_start(dec_sb[:], dec2)
            enc_sb = constp.tile([T, D], F32, tag="enc")
            nc.gpsimd.dma_start(enc_sb[:], enc)
            band_sb = constp.tile([128, 257], BF16, tag="band")
            nc.gpsimd.dma_start(band_sb[:], band)
            onescol_sb = constp.tile([128, 1], BF16, tag="onescol")
            nc.gpsimd.dma_start(onescol_sb[:], onescol)
            ident_sb = constp.tile([128, 128], F32, tag="ident")
            nc.gpsimd.dma_start(ident_sb[:], ident)

            # rowsum^T accumulators: [q (partition), t] for q in [0,128) / [128,256)
            rsT = [
                constp.tile([128, T], F32, tag="rsT_lo", name="rsT_lo"),
                constp.tile([128, T], F32, tag="rsT_hi", name="rsT_hi"),
            ]
            cs_ps = csp.tile([128, D], F32, tag="cs")  # colsum [t, q], PSUM accum
            rr_early = [
                constp.tile([128, T], F32, tag="rr_lo", name="rr_lo_e"),
                constp.tile([128, T], F32, tag="rr_hi", name="rr_hi_e"),
            ]

            mm_idx = 0
            t0 = 0
            for c, tcnt in enumerate(chunk_sizes):
                W = tcnt * D
                bc = bcp.tile([128, TC * D], F32, tag="bc")
                src = enc[t0 : t0 + tcnt, :].rearrange("t d -> (t d)")
                nc.sync.dma_start(bc[:, :W], src.partition_broadcast(128))

                for half in range(2):
                    m_t = mp.tile([128, TC * D], BF16, tag=f"m{half}")
                    scale_ap = dec_sb[:, half : half + 1]
                    a = min(ACCUM_T, tcnt)
                    for tt in range(a):
                        nc.scalar.activation(
                            m_t[:, tt * D : (tt + 1) * D],
                            bc[:, tt * D : (tt + 1) * D],
                            EXP,
                            scale=scale_ap,
                            accum_out=rsT[half][:, t0 + tt : t0 + tt + 1],
                        )
                    if tcnt > a:
                        nc.scalar.activation(
                            m_t[:, a * D : W],
                            bc[:, a * D : W],
                            EXP,
                            scale=scale_ap,
                        )
                        m_v = m_t[:, a * D : W].rearrange(
                            "p (t d) -> p t d", d=D
                        )
                        ft = foldp.tile(
                            [128, TC * (D // 2)], BF16, tag=f"f{half}",
                            name=f"f{half}",
                        )
                        f_v = ft[:, : (tcnt - a) * (D // 2)].rearrange(
                            "p (t d) -> p t d", d=D // 2
                        )
                        nc.vector.tensor_tensor(
                            f_v,
                            m_v[:, :, 0 : D // 2],
                            m_v[:, :, D // 2 : D],
                            op=mybir.AluOpType.add,
                        )
                        nc.vector.tensor_reduce(
                            rsT[half][:, t0 + a : t0 + tcnt],
                            f_v,
                            axis=mybir.AxisListType.X,
                            op=mybir.AluOpType.add,
                        )
                    for tt in range(tcnt):
                        t_abs = t0 + tt
                        nc.tensor.matmul(
                            cs_ps[:],
                            band_sb[:, 128 - t_abs : 256 - t_abs],
                            m_t[:, tt * D : (tt + 1) * D],
                            start=(mm_idx == 0),
                            stop=(mm_idx == n_cs_mms - 1),
                        )
                        mm_idx += 1
                t0 += tcnt

            # ---- epilogue ----
            rr = rr_early
            nc.vector.reciprocal_approx_fast(rr[0][:], rsT[0][:])
            nc.vector.reciprocal_approx_fast(rr[1][:], rsT[1][:])
            tmp = constp.tile([T, D], F32, tag="tmp")
            rrT = constp.tile([T, D], F32, tag="rrT")
            for half in range(2):
                tp = tpp.tile([128, 128], F32, tag="tp")
                nc.tensor.transpose(tp[:], rr[half][:], ident_sb[:])
                sl = slice(half * 128, (half + 1) * 128)
                nc.vector.tensor_copy(rrT[:, sl], tp[:])
                # tmp = (cs_ps * 1.0) * rrT  -- reads cs from PSUM directly
                nc.vector.scalar_tensor_tensor(
                    tmp[:, sl],
                    cs_ps[:, sl],
                    1.0,
                    rrT[:, sl],
                    op0=mybir.AluOpType.mult,
                    op1=mybir.AluOpType.mult,
                )
            contrib = constp.tile([T, D], BF16, tag="contrib")
            nc.vector.tensor_mul(contrib[:], tmp[:], enc_sb[:])
            fin = tpp.tile([1, D], F32, tag="fin")
            nc.tensor.matmul(
                fin[:], onescol_sb[:], contrib[:], start=True, stop=True
            )
            out_sb = constp.tile([1, D], F32, tag="out_sb")
            nc.scalar.copy(out_sb[:], fin[:])
            nc.sync.dma_start(out, out_sb[:])
    nc.compile()
    return nc


_NC_CACHE = None


def _get_nc():
    global _NC_CACHE
    if _NC_CACHE is None:
        _NC_CACHE = build_nc()
    return _NC_CACHE


def make_in_maps(dec_t: np.ndarray, enc_out: np.ndarray):
    band = _band_np()
    ident = _ident_np()
    in_maps = []
    for b in range(B):
        dec2 = np.stack(
            [dec_t[b, :128], dec_t[b, 128:]], axis=1
        ).astype(np.float32)  # [128, 2]
        in_maps.append(
            {
                "dec2": np.ascontiguousarray(dec2),
                "enc": np.ascontiguousarray(enc_out[b]).astype(np.float32),
                "band": band,
                "onescol": np.ones((128, 1), dtype=_band_np().dtype),
                "ident": ident,
            }
        )
    return in_maps


def run(dec_t: np.ndarray, enc_out: np.ndarray, **kwargs):
    """Run on all 8 cores; returns ([B, D] output, BassKernelResults)."""
    nc = _get_nc()
    res = run_bass_kernel_spmd(
        nc, make_in_maps(dec_t, enc_out), core_ids=list(range(NCORES)), **kwargs
    )
    out = np.stack([np.asarray(r["out"]).reshape(D) for r in res.results], axis=0)
    return out.astype(np.float32), res


def kernel(dec_t: np.ndarray, enc_out: np.ndarray) -> np.ndarray:
    dec_t = np.asarray(dec_t, dtype=np.float32)
    enc_out = np.asarray(enc_out, dtype=np.float32)
    out, _ = run(dec_t, enc_out)
    return out



# revision 13
# speedup vs baseline: 1.7223x; 1.7223x over previous
"""Trainium2 Bass kernel for the attention-like exp/reduce problem.

Math (per batch element b, data-parallel across 8 cores):
    colsum[t,q] = sum_p exp(dec[p] * enc[t,q])
    rowsum[t,q] = sum_r exp(dec[q] * enc[t,r])
    out[q]      = sum_t enc[t,q] * colsum[t,q] / rowsum[t,q]

Instead of materializing the [T,D,D] tensor (8.4M exps/core), both sums are
computed from K=16 shared Chebyshev-node exponential tiles via Gaussian-
weighted barycentric interpolation in the dec variable:

    exp(y*x) = e^{y^2/2} * G_x(y),  G_x(y) = e^{y*x - y^2/2}  (Gaussian in y)
    G_x(y) ~= sum_k l_k(y) * G_x(y_k)   (Chebyshev nodes y_k on dec's range;
                                         G_x is entire & bump-shaped -> fast
                                         convergence, benign conditioning)

With E_k[t,q] = exp(y_k*enc[t,q]), cw_k = e^{-y_k^2/2}, V_q = e^{dec_q^2/2},
barycentric weights au[k,q] = w_k/(dec_q - y_k), S_q = sum_k au[k,q]:

    colsum[t,q] ~= sum_k g_k E_k[t,q],   g_k = cw_k * sum_q l_k(dec_q) V_q
    rowsum[t,q] ~= (V_q/S_q) * sum_k (cw_k H[k,t]) au[k,q],  H[k,t] = sum_r E_k[t,r]

Engine split per node k: ACT computes E_k (f32); a single DVE/Pool
tensor_scalar forms GE_k = g_k*E_k with accum_out giving Hg[t,k] = g_k*H[k,t]
for free; PE accumulates colsum = sum_k GE_k in PSUM via an identity
stationary (f32r, 1 cycle/row). Rowsum is one [K,128]x[K,256] matmul of the
cw/g-rescaled transposed Hg against au. S/V and the enc multiply fold into a
precomputed encSV tile; the final t-contraction is a ones-column matmul.
"""

import sys

sys.path.insert(0, "/opt/trn_rl_repo")

import numpy as np

import concourse.bass as bass
import concourse.bacc as bacc
import concourse.tile as tile
from concourse import mybir
from concourse.bass_utils import run_bass_kernel_spmd

# The agent image's antenv package lacks axon_hooks; if BASS_TRACE is set in
# the environment, run_bass_kernel_spmd would die on the import. Provide a
# stub that reports "no hook" so tracing degrades gracefully. (A real hook
# installed earlier, e.g. by prof_shim, is left untouched.)
try:
    import antenv.axon_hooks  # noqa: F401
except ImportError:
    import types

    import antenv

    _hooks = types.ModuleType("antenv.axon_hooks")
    _hooks.get_axon_ntff_profile_hook = lambda: None
    _hooks.set_axon_ntff_profile_hook = lambda h: None
    sys.modules["antenv.axon_hooks"] = _hooks
    antenv.axon_hooks = _hooks

B, T, D = 8, 128, 256
NCORES = 8
K = 16               # Chebyshev nodes (1st kind)
YA, YB = -3.5, 3.5   # node interval (covers the dec value range)
NDV = 10             # nodes with DVE accum-ts; rest: ACT accum + Pool plain ts
F32 = mybir.dt.float32
F32R = mybir.dt.float32r
EXP = mybir.ActivationFunctionType.Exp
ADD = mybir.AluOpType.add
MULT = mybir.AluOpType.mult
DIV = mybir.AluOpType.divide
X_AX = mybir.AxisListType.X


def _nodes():
    j = np.arange(K)
    y = 0.5 * (YA + YB) + 0.5 * (YB - YA) * np.cos((2 * j + 1) * np.pi / (2 * K))
    w = (-1.0) ** j * np.sin((2 * j + 1) * np.pi / (2 * K))
    cw = np.exp(-0.5 * y * y)
    return y.astype(np.float32), w.astype(np.float32), cw.astype(np.float32)


Y_NODES, W_BARY, CW_NODES = _nodes()

# consts_a [128, CA_COLS]:
#   [0:128)        ident
#   [128]          ones column
#   [129:129+K)    nyb  (-y_k broadcast down partitions; per-FREE-col constant)
#   [129+K:129+2K) wb   (w_k broadcast)
#   [129+2K]       nycolv: row k holds -y_k   (per-PARTITION constant)
#   [130+2K]       wcolv:  row k holds  w_k
#   [131+2K]       cwcolv: row k holds cw_k
CO_ONES = 128
CO_NYB = 129
CO_WB = 129 + K
CO_NYCV = 129 + 2 * K
CO_WCV = 130 + 2 * K
CO_CWCV = 131 + 2 * K
CO_MASK = 132 + 2 * K   # 1.0 for k < NDV else 0.0
CO_IMASK = 133 + 2 * K  # 0.0 for k < NDV else 1.0
CA_COLS = 134 + 2 * K
# consts_b [1, 128+K]: onesrow | cwrow
CB_COLS = 128 + K


def _consts_a():
    ca = np.zeros((128, CA_COLS), dtype=np.float32)
    ca[:, :128] = np.eye(128, dtype=np.float32)
    ca[:, CO_ONES] = 1.0
    ca[:, CO_NYB:CO_NYB + K] = -Y_NODES[None, :]
    ca[:, CO_WB:CO_WB + K] = W_BARY[None, :]
    ca[:K, CO_NYCV] = -Y_NODES
    ca[:K, CO_WCV] = W_BARY
    ca[:K, CO_CWCV] = CW_NODES
    ca[:NDV, CO_MASK] = 1.0
    ca[NDV:K, CO_IMASK] = 1.0
    return ca


def _consts_r():
    cr = np.zeros((128, 129), dtype=np.float32)
    cr[:, :128] = np.eye(128, dtype=np.float32)
    cr[:, 128] = 1.0
    return cr


def _consts_br():
    return np.ones((1, 128), dtype=np.float32)


def _consts_b():
    cb = np.zeros((1, CB_COLS), dtype=np.float32)
    cb[0, :128] = 1.0
    cb[0, 128:128 + K] = CW_NODES
    return cb


def build_nc():
    nc = bacc.Bacc("TRN2")
    enc = nc.dram_tensor("enc", [T, D], F32, kind="ExternalInput").ap()
    dec2 = nc.dram_tensor("dec2", [128, 2], F32, kind="ExternalInput").ap()
    decrow = nc.dram_tensor("decrow", [1, D], F32, kind="ExternalInput").ap()
    consts_a = nc.dram_tensor(
        "consts_a", [128, CA_COLS], F32, kind="ExternalInput"
    ).ap()
    consts_b = nc.dram_tensor(
        "consts_b", [1, CB_COLS], F32, kind="ExternalInput"
    ).ap()
    consts_r = nc.dram_tensor(
        "consts_r", [128, 129], F32R, kind="ExternalInput"
    ).ap()
    consts_br = nc.dram_tensor(
        "consts_br", [1, 128], F32R, kind="ExternalInput"
    ).ap()
    out = nc.dram_tensor("out", [1, D], F32, kind="ExternalOutput").ap()

    with tile.TileContext(nc) as tc:
        with (
            tc.tile_pool(name="const", bufs=1) as cp,
            tc.tile_pool(name="e", bufs=4) as ep,
            tc.tile_pool(name="ge", bufs=4) as gep,
            tc.tile_pool(name="cacc", bufs=1, space="PSUM") as caccp,
            tc.tile_pool(name="rs", bufs=1, space="PSUM") as rsp,
            tc.tile_pool(name="svbc", bufs=1, space="PSUM") as svbcp,
            tc.tile_pool(name="psmall", bufs=1, space="PSUM") as psp,

        ):
            # ---- ACT table warmup (no input deps; hides the Exp table load) ----
            warm = cp.tile([128, 1], F32, tag="warm")
            nc.vector.memset(warm[:], 0.0)
            nc.scalar.activation(warm[:], warm[:], EXP)

            # ---- input DMAs ----
            dec_sb = cp.tile([128, 2], F32, tag="dec")
            nc.gpsimd.dma_start(dec_sb[:], dec2)
            ca = cp.tile([128, CA_COLS], F32, tag="ca")
            nc.gpsimd.dma_start(ca[:], consts_a)
            cb = cp.tile([1, CB_COLS], F32, tag="cb")
            nc.gpsimd.dma_start(cb[:], consts_b)
            cr = cp.tile([128, 129], F32R, tag="cr")
            nc.gpsimd.dma_start(cr[:], consts_r)
            cbr = cp.tile([1, 128], F32R, tag="cbr")
            nc.gpsimd.dma_start(cbr[:], consts_br)
            decbc = cp.tile([K, D], F32, tag="decbc")
            nc.gpsimd.dma_start(
                decbc[:], decrow.rearrange("o d -> (o d)").partition_broadcast(K)
            )
            enc_sb = cp.tile([T, D], F32, tag="enc")
            nc.sync.dma_start(enc_sb[:], enc)

            ident = ca[:, 0:128]
            onescol = ca[:, CO_ONES:CO_ONES + 1]
            nyb = ca[:, CO_NYB:CO_NYB + K]
            wb = ca[:, CO_WB:CO_WB + K]
            nycv = ca[0:K, CO_NYCV:CO_NYCV + 1]
            wcv = ca[0:K, CO_WCV:CO_WCV + 1]
            cwcv = ca[0:K, CO_CWCV:CO_CWCV + 1]
            maskcv = ca[0:K, CO_MASK:CO_MASK + 1]
            imaskcv = ca[0:K, CO_IMASK:CO_IMASK + 1]
            onesrow = cb[0:1, 0:128]
            cwrow = cb[0:1, 128:128 + K]
            identr = cr[:, 0:128]
            onescol_r = cr[:, 128:129]
            onesrow_r = cbr[0:1, 0:128]

            # ---- q-partition g chain (tiny ops; feeds gbc for the main loop) ----
            d22 = cp.tile([128, 2], F32, tag="d22")
            nc.vector.tensor_tensor(d22[:], dec_sb[:], dec_sb[:], op=MULT)
            v2 = cp.tile([128, 2], F32, tag="v2")        # e^{+d^2/2}
            nc.scalar.activation(v2[:], d22[:], EXP, scale=0.5)
            vinv2 = cp.tile([128, 2], F32, tag="vinv2")  # e^{-d^2/2}
            nc.scalar.activation(vinv2[:], d22[:], EXP, scale=-0.5)

            dd2 = cp.tile([128, 2 * K], F32, tag="dd2")
            rec2 = cp.tile([128, 2 * K], F32, tag="rec2")
            au2 = cp.tile([128, 2 * K], F32, tag="au2")
            s2 = cp.tile([128, 2], F32, tag="s2")
            srec2 = cp.tile([128, 2], F32, tag="srec2")
            vsc2 = cp.tile([128, 2], F32, tag="vsc2")
            sv2 = cp.tile([128, 2], F32, tag="sv2")
            t5 = cp.tile([128, 2 * K], F32, tag="t5")
            for h in range(2):
                sl = slice(h * K, (h + 1) * K)
                dcol = dec_sb[:, h:h + 1]
                nc.vector.tensor_scalar(dd2[:, sl], nyb, dcol, None, op0=ADD)
                nc.vector.reciprocal_approx_fast(rec2[:, sl], dd2[:, sl])
                nc.vector.tensor_tensor(au2[:, sl], rec2[:, sl], wb, op=MULT)
                nc.vector.tensor_reduce(
                    s2[:, h:h + 1], au2[:, sl], axis=X_AX, op=ADD
                )
            nc.vector.reciprocal_approx_fast(srec2[:], s2[:])
            nc.vector.tensor_tensor(vsc2[:], v2[:], srec2[:], op=MULT)   # V/S
            nc.vector.tensor_tensor(sv2[:], s2[:], vinv2[:], op=MULT)    # S/V
            for h in range(2):
                sl = slice(h * K, (h + 1) * K)
                nc.vector.tensor_scalar(
                    t5[:, sl], au2[:, sl], vsc2[:, h:h + 1], None, op0=MULT
                )
            # shared small-PSUM bank, carved into disjoint regions
            psA = psp.tile([128, 512], F32, tag="psA")
            # g_ps[0, k] = sum_q l_k(dec_q) V_q  (both halves accumulated)
            g_ps = psA[0:1, 0:K]
            nc.tensor.matmul(
                g_ps, onescol, t5[:, 0:K],
                start=True, stop=False,
            )
            nc.tensor.matmul(
                g_ps, onescol, t5[:, K:2 * K],
                start=False, stop=True,
            )
            gsb = cp.tile([1, K], F32, tag="gsb")   # g = cw * (row above)
            nc.vector.tensor_tensor(gsb[:], g_ps, cwrow, op=MULT)
            gbc_ps = psA[:, 16:16 + K]
            nc.tensor.matmul(
                gbc_ps, onesrow, gsb[:],
                start=True, stop=True,
            )
            gbc = cp.tile([128, K], F32, tag="gbcsb")
            nc.vector.tensor_copy(gbc[:], gbc_ps)

            # ---- SV broadcast + encSV (Pool) ----
            svrow = cp.tile([1, D], F32R, tag="svrow")
            for h in range(2):
                svT_ps = psA[0:1, 32 + h * 128:32 + (h + 1) * 128]
                nc.tensor.transpose(svT_ps, sv2[:, h:h + 1], ident)
                nc.vector.tensor_copy(svrow[:, h * 128:(h + 1) * 128], svT_ps)
            svbc_ps = svbcp.tile([128, D], F32, tag="svbc")
            nc.tensor.matmul(
                svbc_ps[:], onesrow_r, svrow[:],
                start=True, stop=True,
            )
            encsv = cp.tile([T, D], F32, tag="encsv")
            nc.vector.tensor_tensor(encsv[:], enc_sb[:], svbc_ps[:], op=MULT)

            # ---- k-partition au (rowsum matmul rhs; off the critical path) ----
            ddk = cp.tile([K, D], F32, tag="ddk")
            reck = cp.tile([K, D], F32, tag="reck")
            auk = cp.tile([K, D], F32, tag="auk")
            nc.vector.tensor_scalar(ddk[:], decbc[:], nycv, None, op0=ADD)
            nc.vector.reciprocal_approx_fast(reck[:], ddk[:])
            nc.vector.tensor_scalar(auk[:], reck[:], wcv, None, op0=MULT)

            # cwg: cw_k/g_k for DVE-accum nodes (Hg holds g*H); cw_k for
            # ACT-accum nodes (Hg holds plain H)
            gT_ps = psA[0:K, 288:289]
            nc.tensor.transpose(gT_ps, gsb[:], ident[0:1, 0:1])
            gcol = cp.tile([K, 1], F32, tag="gcol")
            nc.vector.tensor_copy(gcol[:], gT_ps)
            grec = cp.tile([K, 1], F32, tag="grec")
            nc.vector.reciprocal_approx_fast(grec[:], gcol[:])
            gsel = cp.tile([K, 1], F32, tag="gsel")
            nc.vector.tensor_scalar(gsel[:], grec[:], maskcv, None, op0=MULT)
            gsel2 = cp.tile([K, 1], F32, tag="gsel2")
            nc.vector.tensor_tensor(gsel2[:], gsel[:], imaskcv, op=ADD)
            cwg = cp.tile([K, 1], F32, tag="cwg")
            nc.vector.tensor_scalar(cwg[:], gsel2[:], cwcv, None, op0=MULT)

            # ---- main loop: E_k -> GE_k (+Hg accum) -> PSUM colsum accumulate ----
            hg = cp.tile([128, K], F32, tag="hg")
            cacc_ps = caccp.tile([T, D], F32, tag="cacc")
            for k in range(K):
                e_t = ep.tile([T, D], F32, tag="e")
                if k < NDV:
                    nc.scalar.activation(
                        e_t[:], enc_sb[:], EXP, scale=float(Y_NODES[k])
                    )
                else:
                    # ACT accumulates plain H for this node
                    nc.scalar.activation(
                        e_t[:], enc_sb[:], EXP, scale=float(Y_NODES[k]),
                        accum_out=hg[:, k:k + 1],
                    )
                ge_t = gep.tile([T, D], F32R, tag="ge")
                if k < NDV:
                    nc.vector.tensor_scalar(
                        ge_t[:], e_t[:], gbc[:, k:k + 1], 0.0, op0=MULT, op1=ADD,
                        accum_out=hg[:, k:k + 1],
                    )
                else:
                    nc.gpsimd.tensor_scalar(
                        ge_t[:], e_t[:], gbc[:, k:k + 1], None, op0=MULT
                    )
                nc.tensor.matmul(
                    cacc_ps[:], identr, ge_t[:],
                    start=(k == 0), stop=(k == K - 1),
                )

            # ---- rowsum interp + combine ----
            hgT_ps = psA[0:K, 289:417]
            nc.tensor.transpose(hgT_ps, hg[:], ident)
            htw = cp.tile([K, 128], F32, tag="htw")
            nc.vector.tensor_scalar(htw[:], hgT_ps, cwg[:], None, op0=MULT)
            rsA = rsp.tile([128, 512], F32, tag="rsA")
            rs_ps = rsA[:, 0:256]
            nc.tensor.matmul(
                rs_ps, htw[:], auk[:],
                start=True, stop=True,
            )
            rrec = cp.tile([T, D], F32, tag="rrec")
            nc.vector.reciprocal_approx_fast(rrec[:], rs_ps)
            c1 = cp.tile([T, D], F32, tag="c1")
            nc.vector.scalar_tensor_tensor(
                c1[:], cacc_ps[:], 1.0, rrec[:], op0=MULT, op1=MULT
            )
            contrib = cp.tile([T, D], F32R, tag="contrib")
            nc.vector.tensor_tensor(contrib[:], c1[:], encsv[:], op=MULT)
            fin_ps = rsA[0:1, 256:512]
            nc.tensor.matmul(
                fin_ps, onescol_r, contrib[:],
                start=True, stop=True,
            )
            out_sb = cp.tile([1, D], F32, tag="outsb")
            nc.vector.tensor_copy(out_sb[:], fin_ps)
            nc.sync.dma_start(out, out_sb[:])
    nc.compile()
    return nc


_NC_CACHE = None


def _get_nc():
    global _NC_CACHE
    if _NC_CACHE is None:
        _NC_CACHE = build_nc()
    return _NC_CACHE


def make_in_maps(dec_t: np.ndarray, enc_out: np.ndarray):
    ca = _consts_a()
    cb = _consts_b()
    cr = _consts_r()
    cbr = _consts_br()
    in_maps = []
    for b in range(B):
        dec2 = np.stack([dec_t[b, :128], dec_t[b, 128:]], axis=1)
        in_maps.append(
            {
                "enc": np.ascontiguousarray(enc_out[b]).astype(np.float32),
                "dec2": np.ascontiguousarray(dec2).astype(np.float32),
                "decrow": np.ascontiguousarray(dec_t[b][None, :]).astype(np.float32),
                "consts_a": ca,
                "consts_b": cb,
                "consts_r": cr,
                "consts_br": cbr,
            }
        )
    return in_maps


def run(dec_t: np.ndarray, enc_out: np.ndarray, **kwargs):
    """Run on all 8 cores; returns ([B, D] output, BassKernelResults)."""
    nc = _get_nc()
    res = run_bass_kernel_spmd(
        nc, make_in_maps(dec_t, enc_out), core_ids=list(range(NCORES)), **kwargs
    )
    out = np.stack([np.asarray(r["out"]).reshape(D) for r in res.results], axis=0)
    return out.astype(np.float32), res


def kernel(dec_t: np.ndarray, enc_out: np.ndarray) -> np.ndarray:
    dec_t = np.asarray(dec_t, dtype=np.float32)
    enc_out = np.asarray(enc_out, dtype=np.float32)
    out, _ = run(dec_t, enc_out)
    return out


# revision 18
# speedup vs baseline: 2.9824x; 1.7317x over previous
"""Trainium2 Bass kernel for the attention-like exp/reduce problem.

Math (per batch element b, data-parallel across 8 cores):
    colsum[t,q] = sum_p exp(dec[p] * enc[t,q])
    rowsum[t,q] = sum_r exp(dec[q] * enc[t,r])
    out[q]      = sum_t enc[t,q] * colsum[t,q] / rowsum[t,q]

Instead of materializing the [T,D,D] tensor (8.4M exps/core), both sums are
computed from K=14 shared Chebyshev-node exponential tiles via Gaussian-
weighted barycentric interpolation in the dec variable:

    exp(y*x) = e^{y^2/2} * G_x(y),  G_x(y) = e^{y*x - y^2/2}  (Gaussian in y)
    G_x(y) ~= sum_k l_k(y) * G_x(y_k)   (Chebyshev nodes y_k on dec's range;
                                         G_x is entire & bump-shaped -> fast
                                         convergence, benign conditioning)

With E_k[t,q] = exp(y_k*enc[t,q]), cw_k = e^{-y_k^2/2}, V_q = e^{dec_q^2/2},
barycentric weights au[k,q] = w_k/(dec_q - y_k), S_q = sum_k au[k,q]:

    colsum[t,q] ~= sum_k g_k E_k[t,q],   g_k = cw_k * sum_q l_k(dec_q) V_q
    rowsum[t,q] ~= (V_q/S_q) * sum_k (cw_k H[k,t]) au[k,q],  H[k,t] = sum_r E_k[t,r]

Engine split per node k: ACT computes E_k (f32).  For the first NDV nodes a
single DVE tensor_scalar forms GE_k = g_k*E_k (f32r) with accum_out giving
Hg[t,k] = g_k*H[k,t] for free; for the rest, H comes from the ACT accum_out
and DVE only does the plain g-scale (the cw/g vs cw rescale is folded into a
masked cwg column).  PE accumulates colsum = sum_k GE_k in PSUM via an
identity stationary (f32r, single-pass).  Rowsum is one [K,128]x[K,256] f32
matmul of the rescaled transposed Hg against au.  S/V and the enc multiply
fold into a precomputed encSV tile; the final t-contraction is a ones-column
matmul.  GPSIMD/Pool does no compute (its tensor ops run ~4us each on HW).
"""

import sys

sys.path.insert(0, "/opt/trn_rl_repo")

import numpy as np

import concourse.bacc as bacc
import concourse.tile as tile
from concourse import mybir
from concourse.bass_utils import run_bass_kernel_spmd

try:
    import antenv.axon_hooks  # noqa: F401
except ImportError:
    import types

    import antenv

    _hooks = types.ModuleType("antenv.axon_hooks")
    _hooks.get_axon_ntff_profile_hook = lambda: None
    _hooks.set_axon_ntff_profile_hook = lambda h: None
    sys.modules["antenv.axon_hooks"] = _hooks
    antenv.axon_hooks = _hooks

B, T, D = 8, 128, 256
NCORES = 8
K = 14               # Chebyshev nodes (1st kind)
NDV = 8              # nodes whose H rides the DVE accum-ts; rest use ACT accum
YA, YB = -3.5, 3.5   # node interval (covers the dec value range)
F32 = mybir.dt.float32
F32R = mybir.dt.float32r
EXP = mybir.ActivationFunctionType.Exp
ADD = mybir.AluOpType.add
MULT = mybir.AluOpType.mult
X_AX = mybir.AxisListType.X


def _nodes():
    j = np.arange(K)
    y = 0.5 * (YA + YB) + 0.5 * (YB - YA) * np.cos((2 * j + 1) * np.pi / (2 * K))
    w = (-1.0) ** j * np.sin((2 * j + 1) * np.pi / (2 * K))
    cw = np.exp(-0.5 * y * y)
    return y.astype(np.float32), w.astype(np.float32), cw.astype(np.float32)


Y_NODES, W_BARY, CW_NODES = _nodes()

# blob_f [128, CF]: f32 constants
#   [0:128)  ident          [128] onescol        [129:129+K) nyb (-y bcast)
#   [129+K:129+2K) wb       [129+2K] nycv        [130+2K] wcv
#   [131+2K] cwcv           [132+2K] mask(k<NDV) [133+2K] imask
#   [134+2K:134+3K) onesrowK (row 0 only = 1)
CF_ONES = 128
CF_NYB = 129
CF_WB = 129 + K
CF_NYCV = 129 + 2 * K
CF_WCV = 130 + 2 * K
CF_CWCV = 131 + 2 * K
CF_MASK = 132 + 2 * K
CF_IMASK = 133 + 2 * K
CF_ONESROWK = 134 + 2 * K
CF_ONESROW = 134 + 3 * K
CF_COLS = 262 + 3 * K

# blob_r [128, CR]: f32r constants
#   [0:128) identr   [128] onescolr   [129:257) onesrow (row 0 = 1)
#   [257:257+K) cwrow (row 0 = cw)
CR_ONESCOL = 128
CR_ONESROW = 129
CR_CWROW = 257
CR_COLS = 257 + K


def _blob_f():
    bf = np.zeros((128, CF_COLS), dtype=np.float32)
    bf[:, :128] = np.eye(128, dtype=np.float32)
    bf[:, CF_ONES] = 1.0
    bf[:, CF_NYB:CF_NYB + K] = -Y_NODES[None, :]
    bf[:, CF_WB:CF_WB + K] = W_BARY[None, :]
    bf[:K, CF_NYCV] = -Y_NODES
    bf[:K, CF_WCV] = W_BARY
    bf[:K, CF_CWCV] = CW_NODES
    bf[:NDV, CF_MASK] = 1.0
    bf[NDV:K, CF_IMASK] = 1.0
    bf[0, CF_ONESROWK:CF_ONESROWK + K] = 1.0
    bf[0, CF_ONESROW:CF_ONESROW + 128] = 1.0
    return bf


def _blob_r():
    br = np.zeros((128, CR_COLS), dtype=np.float32)
    br[:, :128] = np.eye(128, dtype=np.float32)
    br[:, CR_ONESCOL] = 1.0
    br[0, CR_ONESROW:CR_ONESROW + 128] = 1.0
    br[0, CR_CWROW:CR_CWROW + K] = CW_NODES
    return br


def build_nc():
    nc = bacc.Bacc("TRN2")
    # enc_x: enc columns 0:256, dec2 columns 256:258
    enc_x = nc.dram_tensor("enc_x", [T, D + 2], F32, kind="ExternalInput").ap()
    decrow = nc.dram_tensor("decrow", [1, D], F32, kind="ExternalInput").ap()
    blob_f = nc.dram_tensor("blob_f", [128, CF_COLS], F32, kind="ExternalInput").ap()
    blob_r = nc.dram_tensor("blob_r", [128, CR_COLS], F32R, kind="ExternalInput").ap()
    out = nc.dram_tensor("out", [1, D], F32, kind="ExternalOutput").ap()

    with tile.TileContext(nc) as tc:
        with (
            tc.tile_pool(name="const", bufs=1) as cp,
            tc.tile_pool(name="e", bufs=6) as ep,
            tc.tile_pool(name="ge", bufs=4) as gep,
            tc.tile_pool(name="cacc", bufs=1, space="PSUM") as caccp,
            tc.tile_pool(name="rsb", bufs=1, space="PSUM") as rsp,
            tc.tile_pool(name="svbc", bufs=1, space="PSUM") as svbcp,
            tc.tile_pool(name="psmall", bufs=1, space="PSUM") as psp,
        ):
            # ---- ACT table warmup (no input deps; hides the Exp table load) ----
            warm = cp.tile([128, 1], F32, tag="warm")
            nc.vector.memset(warm[:], 0.0)
            nc.scalar.activation(warm[:], warm[:], EXP)

            # ---- input DMAs, issue spread across engines ----
            encx_sb = cp.tile([T, D + 2], F32, tag="encx")
            nc.sync.dma_start(encx_sb[:], enc_x)
            bf = cp.tile([128, CF_COLS], F32, tag="bf")
            nc.gpsimd.dma_start(bf[:], blob_f)
            br = cp.tile([128, CR_COLS], F32R, tag="br")
            nc.scalar.dma_start(br[:], blob_r)
            drow = cp.tile([1, D], F32, tag="drow")
            nc.sync.dma_start(drow[:], decrow)

            enc_sb = encx_sb[:, 0:D]
            dec_sb = encx_sb[:, D:D + 2]
            ident = bf[:, 0:128]
            onescol = bf[:, CF_ONES:CF_ONES + 1]
            nyb = bf[:, CF_NYB:CF_NYB + K]
            wb = bf[:, CF_WB:CF_WB + K]
            nycv = bf[0:K, CF_NYCV:CF_NYCV + 1]
            wcv = bf[0:K, CF_WCV:CF_WCV + 1]
            cwcv = bf[0:K, CF_CWCV:CF_CWCV + 1]
            maskcv = bf[0:K, CF_MASK:CF_MASK + 1]
            imaskcv = bf[0:K, CF_IMASK:CF_IMASK + 1]
            onesrowK = bf[0:1, CF_ONESROWK:CF_ONESROWK + K]
            onesrow_f = bf[0:1, CF_ONESROW:CF_ONESROW + 128]
            identr = br[:, 0:128]
            onescol_r = br[:, CR_ONESCOL:CR_ONESCOL + 1]
            onesrow_r = br[0:1, CR_ONESROW:CR_ONESROW + 128]
            cwrow_r = br[0:1, CR_CWROW:CR_CWROW + K]

            # ---- q-partition g chain (tiny ops; feeds gbc for the main loop) ----
            d22 = cp.tile([128, 2], F32, tag="d22")
            nc.vector.tensor_tensor(d22[:], dec_sb, dec_sb, op=MULT)
            nd22 = cp.tile([128, 4], F32, tag="nd22")
            nc.vector.tensor_copy(nd22[:, 0:2], d22[:])
            nc.vector.tensor_scalar(nd22[:, 2:4], d22[:], -1.0, None, op0=MULT)
            vv = cp.tile([128, 4], F32, tag="vv")  # [e^{+d^2/2} | e^{-d^2/2}]
            nc.scalar.activation(vv[:], nd22[:], EXP, scale=0.5)
            v2 = vv[:, 0:2]
            vinv2 = vv[:, 2:4]

            dd2 = cp.tile([128, 2 * K], F32, tag="dd2")
            rec2 = cp.tile([128, 2 * K], F32, tag="rec2")
            au2 = cp.tile([128, 2 * K], F32, tag="au2")
            s2 = cp.tile([128, 2], F32, tag="s2")
            srec2 = cp.tile([128, 2], F32, tag="srec2")
            vsc2 = cp.tile([128, 2], F32, tag="vsc2")
            sv2 = cp.tile([128, 2], F32, tag="sv2")
            t5 = cp.tile([128, 2 * K], F32R, tag="t5")
            for h in range(2):
                sl = slice(h * K, (h + 1) * K)
                dcol = dec_sb[:, h:h + 1]
                nc.vector.tensor_scalar(dd2[:, sl], nyb, dcol, None, op0=ADD)
                nc.vector.reciprocal_approx_fast(rec2[:, sl], dd2[:, sl])
                nc.vector.tensor_tensor(au2[:, sl], rec2[:, sl], wb, op=MULT)
                nc.vector.tensor_reduce(
                    s2[:, h:h + 1], au2[:, sl], axis=X_AX, op=ADD
                )
            nc.vector.reciprocal_approx_fast(srec2[:], s2[:])
            nc.vector.tensor_tensor(vsc2[:], v2, srec2[:], op=MULT)   # V/S
            nc.vector.tensor_tensor(sv2[:], s2[:], vinv2, op=MULT)    # S/V
            for h in range(2):
                sl = slice(h * K, (h + 1) * K)
                nc.vector.tensor_scalar(
                    t5[:, sl], au2[:, sl], vsc2[:, h:h + 1], None, op0=MULT
                )
            # g_ps[0, k] = sum_q l_k(dec_q) V_q  (both halves accumulated)
            psA = psp.tile([128, 512], F32, tag="psA")
            g_ps = psA[0:1, 0:K]
            nc.tensor.matmul(g_ps, onescol_r, t5[:, 0:K], start=True, stop=False)
            nc.tensor.matmul(g_ps, onescol_r, t5[:, K:2 * K], start=False, stop=True)
            gsb = cp.tile([1, K], F32, tag="gsb")   # g = cw * (row above)
            nc.vector.tensor_tensor(gsb[:], g_ps, cwrow_r, op=MULT)
            gbc_ps = psA[:, 16:16 + K]
            nc.tensor.matmul(gbc_ps, onesrow_f, gsb[:], start=True, stop=True)
            gbc = cp.tile([128, K], F32, tag="gbcsb")
            nc.vector.tensor_copy(gbc[:], gbc_ps)

            # ---- SV broadcast + encSV ----
            svrow = cp.tile([1, D], F32R, tag="svrow")
            for h in range(2):
                svT_ps = psA[0:1, 32 + h * 128:32 + (h + 1) * 128]
                nc.tensor.transpose(svT_ps, sv2[:, h:h + 1], ident)
                nc.vector.tensor_copy(svrow[:, h * 128:(h + 1) * 128], svT_ps)
            svbc_ps = svbcp.tile([128, D], F32, tag="svbc")
            nc.tensor.matmul(
                svbc_ps[:], onesrow_r, svrow[:], start=True, stop=True
            )
            encsv = cp.tile([T, D], F32, tag="encsv")
            nc.vector.tensor_tensor(encsv[:], enc_sb, svbc_ps[:], op=MULT)

            # ---- k-partition au (rowsum matmul rhs); decbc via PE outer ----
            decbc_ps = psp.tile([K, 256], F32, tag="psB")
            nc.tensor.matmul(decbc_ps[:], onesrowK, drow[:], start=True, stop=True)
            ddk = cp.tile([K, D], F32, tag="ddk")
            nc.vector.tensor_scalar(ddk[:], decbc_ps[:], nycv, None, op0=ADD)
            reck = cp.tile([K, D], F32, tag="reck")
            nc.vector.reciprocal_approx_fast(reck[:], ddk[:])
            auk = cp.tile([K, D], F32, tag="auk")
            nc.vector.tensor_scalar(auk[:], reck[:], wcv, None, op0=MULT)

            # cwg: cw/g for DVE-accum nodes (Hg holds g*H), cw for ACT-accum
            # nodes (Hg holds plain H); selected via constant masks.
            gT_ps = psA[0:K, 288:289]
            nc.tensor.transpose(gT_ps, gsb[:], ident[0:1, 0:1])
            gcol = cp.tile([K, 1], F32, tag="gcol")
            nc.vector.tensor_copy(gcol[:], gT_ps)
            grec = cp.tile([K, 1], F32, tag="grec")
            nc.vector.reciprocal_approx_fast(grec[:], gcol[:])
            gsel = cp.tile([K, 1], F32, tag="gsel")
            nc.vector.tensor_scalar(gsel[:], grec[:], maskcv, None, op0=MULT)
            gsel2 = cp.tile([K, 1], F32, tag="gsel2")
            nc.vector.tensor_tensor(gsel2[:], gsel[:], imaskcv, op=ADD)
            cwg = cp.tile([K, 1], F32, tag="cwg")
            nc.vector.tensor_scalar(cwg[:], gsel2[:], cwcv, None, op0=MULT)

            # ---- main loop: E_k -> GE_k (+H accum) -> PSUM colsum accumulate ----
            hg = cp.tile([128, K], F32, tag="hg")
            cacc_ps = caccp.tile([T, D], F32, tag="cacc")
            for k in range(K):
                e_t = ep.tile([T, D], F32, tag="e")
                if k < NDV:
                    nc.scalar.activation(
                        e_t[:], enc_sb, EXP, scale=float(Y_NODES[k])
                    )
                else:
                    nc.scalar.activation(
                        e_t[:], enc_sb, EXP, scale=float(Y_NODES[k]),
                        accum_out=hg[:, k:k + 1],
                    )
                ge_t = gep.tile([T, D], F32R, tag="ge")
                if k < NDV:
                    nc.vector.tensor_scalar(
                        ge_t[:], e_t[:], gbc[:, k:k + 1], 0.0, op0=MULT, op1=ADD,
                        accum_out=hg[:, k:k + 1],
                    )
                else:
                    nc.vector.tensor_scalar(
                        ge_t[:], e_t[:], gbc[:, k:k + 1], None, op0=MULT
                    )
                nc.tensor.matmul(
                    cacc_ps[:], identr, ge_t[:],
                    start=(k == 0), stop=(k == K - 1),
                )

            # ---- rowsum interp + combine ----
            hgT_ps = psA[0:K, 384:512]
            nc.tensor.transpose(hgT_ps, hg[:], ident)
            htw = cp.tile([K, 128], F32, tag="htw")
            nc.vector.tensor_scalar(htw[:], hgT_ps, cwg[:], None, op0=MULT)
            rsA = rsp.tile([128, 512], F32, tag="rsA")
            rs_ps = rsA[:, 0:256]
            nc.tensor.matmul(rs_ps, htw[:], auk[:], start=True, stop=True)
            rrec = cp.tile([T, D], F32, tag="rrec")
            nc.vector.reciprocal_approx_fast(rrec[:], rs_ps)
            c1 = cp.tile([T, D], F32, tag="c1")
            nc.vector.scalar_tensor_tensor(
                c1[:], cacc_ps[:], 1.0, rrec[:], op0=MULT, op1=MULT
            )
            contrib = cp.tile([T, D], F32R, tag="contrib")
            nc.vector.tensor_tensor(contrib[:], c1[:], encsv[:], op=MULT)
            fin_ps = rsA[0:1, 256:512]
            nc.tensor.matmul(fin_ps, onescol_r, contrib[:], start=True, stop=True)
            out_sb = cp.tile([1, D], F32, tag="outsb")
            nc.vector.tensor_copy(out_sb[:], fin_ps)
            nc.sync.dma_start(out, out_sb[:])
    nc.compile()
    return nc


_NC_CACHE = None


def _get_nc():
    global _NC_CACHE
    if _NC_CACHE is None:
        _NC_CACHE = build_nc()
    return _NC_CACHE


def make_in_maps(dec_t: np.ndarray, enc_out: np.ndarray):
    bf = _blob_f()
    br = _blob_r()
    in_maps = []
    for b in range(B):
        dec2 = np.stack([dec_t[b, :128], dec_t[b, 128:]], axis=1)
        enc_x = np.concatenate([enc_out[b], dec2], axis=1).astype(np.float32)
        in_maps.append(
            {
                "enc_x": np.ascontiguousarray(enc_x),
                "decrow": np.ascontiguousarray(
                    dec_t[b][None, :]
                ).astype(np.float32),
                "blob_f": bf,
                "blob_r": br,
            }
        )
    return in_maps


def run(dec_t: np.ndarray, enc_out: np.ndarray, **kwargs):
    """Run on all 8 cores; returns ([B, D] output, BassKernelResults)."""
    nc = _get_nc()
    res = run_bass_kernel_spmd(
        nc, make_in_maps(dec_t, enc_out), core_ids=list(range(NCORES)), **kwargs
    )
    out = np.stack([np.asarray(r["out"]).reshape(D) for r in res.results], axis=0)
    return out.astype(np.float32), res


def kernel(dec_t: np.ndarray, enc_out: np.ndarray) -> np.ndarray:
    dec_t = np.asarray(dec_t, dtype=np.float32)
    enc_out = np.asarray(enc_out, dtype=np.float32)
    out, _ = run(dec_t, enc_out)
    return out


# revision 22
# speedup vs baseline: 2.9849x; 1.0008x over previous
"""Trainium2 Bass kernel for the attention-like exp/reduce problem.

Math (per batch element b, data-parallel across 8 cores):
    colsum[t,q] = sum_p exp(dec[p] * enc[t,q])
    rowsum[t,q] = sum_r exp(dec[q] * enc[t,r])
    out[q]      = sum_t enc[t,q] * colsum[t,q] / rowsum[t,q]

Instead of materializing the [T,D,D] tensor (8.4M exps/core), both sums are
computed from K=14 shared Chebyshev-node exponential tiles via Gaussian-
weighted barycentric interpolation in the dec variable:

    exp(y*x) = e^{y^2/2} * G_x(y),  G_x(y) = e^{y*x - y^2/2}  (Gaussian in y)
    G_x(y) ~= sum_k l_k(y) * G_x(y_k)   (Chebyshev nodes y_k on dec's range;
                                         G_x is entire & bump-shaped -> fast
                                         convergence, benign conditioning)

With E_k[t,q] = exp(y_k*enc[t,q]), cw_k = e^{-y_k^2/2}, V_q = e^{dec_q^2/2},
barycentric weights au[k,q] = w_k/(dec_q - y_k), S_q = sum_k au[k,q]:

    colsum[t,q] ~= sum_k g_k E_k[t,q],   g_k = cw_k * sum_q l_k(dec_q) V_q
    rowsum[t,q] ~= (V_q/S_q) * sum_k (cw_k H[k,t]) au[k,q],  H[k,t] = sum_r E_k[t,r]

Engine split per node k: ACT computes E_k (f32).  For the first NDV nodes a
single DVE tensor_scalar forms GE_k = g_k*E_k (f32r) with accum_out giving
Hg[t,k] = g_k*H[k,t] for free; for the rest, H comes from the ACT accum_out
and DVE only does the plain g-scale (the cw/g vs cw rescale is folded into a
masked cwg column).  PE accumulates colsum = sum_k GE_k in PSUM via an
identity stationary (f32r, single-pass).  Rowsum is one [K,128]x[K,256] f32
matmul of the rescaled transposed Hg against au.  S/V and the enc multiply
fold into a precomputed encSV tile; the final t-contraction is a ones-column
matmul.  GPSIMD/Pool does no compute (its tensor ops run ~4us each on HW).
"""

import sys

sys.path.insert(0, "/opt/trn_rl_repo")

import numpy as np

import concourse.bacc as bacc
import concourse.tile as tile
from concourse import mybir
from concourse.bass_utils import run_bass_kernel_spmd

try:
    import antenv.axon_hooks  # noqa: F401
except ImportError:
    import types

    import antenv

    _hooks = types.ModuleType("antenv.axon_hooks")
    _hooks.get_axon_ntff_profile_hook = lambda: None
    _hooks.set_axon_ntff_profile_hook = lambda h: None
    sys.modules["antenv.axon_hooks"] = _hooks
    antenv.axon_hooks = _hooks

B, T, D = 8, 128, 256
NCORES = 8
K = 14               # Chebyshev nodes (1st kind)
NDV = 9              # nodes whose H rides the DVE accum-ts; rest use ACT accum
YA, YB = -3.5, 3.5   # node interval (covers the dec value range)
F32 = mybir.dt.float32
F32R = mybir.dt.float32r
EXP = mybir.ActivationFunctionType.Exp
ADD = mybir.AluOpType.add
MULT = mybir.AluOpType.mult
X_AX = mybir.AxisListType.X


def _nodes():
    j = np.arange(K)
    y = 0.5 * (YA + YB) + 0.5 * (YB - YA) * np.cos((2 * j + 1) * np.pi / (2 * K))
    w = (-1.0) ** j * np.sin((2 * j + 1) * np.pi / (2 * K))
    cw = np.exp(-0.5 * y * y)
    return y.astype(np.float32), w.astype(np.float32), cw.astype(np.float32)


Y_NODES, W_BARY, CW_NODES = _nodes()

# blob_f [128, CF]: f32 constants
#   [0:128)  ident          [128] onescol        [129:129+K) nyb (-y bcast)
#   [129+K:129+2K) wb       [129+2K] nycv        [130+2K] wcv
#   [131+2K] cwcv           [132+2K] mask(k<NDV) [133+2K] imask
#   [134+2K:134+3K) onesrowK (row 0 only = 1)
CF_ONES = 128
CF_NYB = 129
CF_WB = 129 + K
CF_NYCV = 129 + 2 * K
CF_WCV = 130 + 2 * K
CF_CWCV = 131 + 2 * K
CF_MASK = 132 + 2 * K
CF_IMASK = 133 + 2 * K
CF_ONESROWK = 134 + 2 * K
CF_ONESROW = 134 + 3 * K
CF_COLS = 262 + 3 * K

# blob_r [128, CR]: f32r constants
#   [0:128) identr   [128] onescolr   [129:257) onesrow (row 0 = 1)
#   [257:257+K) cwrow (row 0 = cw)
CR_ONESCOL = 128
CR_ONESROW = 129
CR_CWROW = 257
CR_COLS = 257 + K


def _blob_f():
    bf = np.zeros((128, CF_COLS), dtype=np.float32)
    bf[:, :128] = np.eye(128, dtype=np.float32)
    bf[:, CF_ONES] = 1.0
    bf[:, CF_NYB:CF_NYB + K] = -Y_NODES[None, :]
    bf[:, CF_WB:CF_WB + K] = W_BARY[None, :]
    bf[:K, CF_NYCV] = -Y_NODES
    bf[:K, CF_WCV] = W_BARY
    bf[:K, CF_CWCV] = CW_NODES
    bf[:NDV, CF_MASK] = 1.0
    bf[NDV:K, CF_IMASK] = 1.0
    bf[0, CF_ONESROWK:CF_ONESROWK + K] = 1.0
    bf[0, CF_ONESROW:CF_ONESROW + 128] = 1.0
    return bf


def _blob_r():
    br = np.zeros((128, CR_COLS), dtype=np.float32)
    br[:, :128] = np.eye(128, dtype=np.float32)
    br[:, CR_ONESCOL] = 1.0
    br[0, CR_ONESROW:CR_ONESROW + 128] = 1.0
    br[0, CR_CWROW:CR_CWROW + K] = CW_NODES
    return br


def build_nc():
    nc = bacc.Bacc("TRN2")
    enc = nc.dram_tensor("enc", [T, D], F32, kind="ExternalInput").ap()
    decq = nc.dram_tensor("decq", [128, 2], F32, kind="ExternalInput").ap()
    decrow = nc.dram_tensor("decrow", [1, D], F32, kind="ExternalInput").ap()
    blob_f = nc.dram_tensor("blob_f", [128, CF_COLS], F32, kind="ExternalInput").ap()
    blob_r = nc.dram_tensor("blob_r", [128, CR_COLS], F32R, kind="ExternalInput").ap()
    out = nc.dram_tensor("out", [1, D], F32, kind="ExternalOutput").ap()

    with tile.TileContext(nc) as tc:
        with (
            tc.tile_pool(name="const", bufs=1) as cp,
            tc.tile_pool(name="e", bufs=10) as ep,
            tc.tile_pool(name="ge", bufs=4) as gep,
            tc.tile_pool(name="cacc", bufs=1, space="PSUM") as caccp,
            tc.tile_pool(name="rsb", bufs=1, space="PSUM") as rsp,
            tc.tile_pool(name="svbc", bufs=1, space="PSUM") as svbcp,
            tc.tile_pool(name="psmall", bufs=1, space="PSUM") as psp,
        ):
            # ---- ACT table warmup (no input deps; hides the Exp table load).
            # zt stays all-zero and doubles as an explicit bias AP for every
            # activation, avoiding a late const-tensor dependency. ----
            zt = cp.tile([128, 1], F32, tag="zt")
            nc.vector.memset(zt[:], 0.0)
            warm = cp.tile([128, 1], F32, tag="warm")
            nc.scalar.activation(warm[:], zt[:], EXP, bias=zt[:, 0:1])

            # ---- input DMAs: tiny dec tensors first, spread across engines ----
            dec_t2 = cp.tile([128, 2], F32, tag="decq")
            nc.sync.dma_start(dec_t2[:], decq)
            drow = cp.tile([1, D], F32, tag="drow")
            nc.sync.dma_start(drow[:], decrow)
            enc_sb = cp.tile([T, D], F32, tag="enc")
            nc.sync.dma_start(enc_sb[:], enc)
            bf = cp.tile([128, CF_COLS], F32, tag="bf")
            nc.gpsimd.dma_start(bf[:], blob_f)
            br = cp.tile([128, CR_COLS], F32R, tag="br")
            nc.scalar.dma_start(br[:], blob_r)

            dec_sb = dec_t2[:]
            ident = bf[:, 0:128]
            onescol = bf[:, CF_ONES:CF_ONES + 1]
            nyb = bf[:, CF_NYB:CF_NYB + K]
            wb = bf[:, CF_WB:CF_WB + K]
            nycv = bf[0:K, CF_NYCV:CF_NYCV + 1]
            wcv = bf[0:K, CF_WCV:CF_WCV + 1]
            cwcv = bf[0:K, CF_CWCV:CF_CWCV + 1]
            maskcv = bf[0:K, CF_MASK:CF_MASK + 1]
            imaskcv = bf[0:K, CF_IMASK:CF_IMASK + 1]
            onesrowK = bf[0:1, CF_ONESROWK:CF_ONESROWK + K]
            onesrow_f = bf[0:1, CF_ONESROW:CF_ONESROW + 128]
            identr = br[:, 0:128]
            onescol_r = br[:, CR_ONESCOL:CR_ONESCOL + 1]
            onesrow_r = br[0:1, CR_ONESROW:CR_ONESROW + 128]
            cwrow_r = br[0:1, CR_CWROW:CR_CWROW + K]

            # ---- q-partition g chain (tiny ops; feeds gbc for the main loop) ----
            d22 = cp.tile([128, 2], F32, tag="d22")
            nc.vector.tensor_tensor(d22[:], dec_sb, dec_sb, op=MULT)
            nd22 = cp.tile([128, 4], F32, tag="nd22")
            nc.vector.tensor_copy(nd22[:, 0:2], d22[:])
            nc.vector.tensor_scalar(nd22[:, 2:4], d22[:], -1.0, None, op0=MULT)
            vv = cp.tile([128, 4], F32, tag="vv")  # [e^{+d^2/2} | e^{-d^2/2}]
            nc.scalar.activation(vv[:], nd22[:], EXP, scale=0.5, bias=zt[:, 0:1])
            v2 = vv[:, 0:2]
            vinv2 = vv[:, 2:4]

            dd2 = cp.tile([128, 2 * K], F32, tag="dd2")
            rec2 = cp.tile([128, 2 * K], F32, tag="rec2")
            au2 = cp.tile([128, 2 * K], F32, tag="au2")
            s2 = cp.tile([128, 2], F32, tag="s2")
            srec2 = cp.tile([128, 2], F32, tag="srec2")
            vsc2 = cp.tile([128, 2], F32, tag="vsc2")
            sv2 = cp.tile([128, 2], F32, tag="sv2")
            t5 = cp.tile([128, 2 * K], F32R, tag="t5")
            for h in range(2):
                sl = slice(h * K, (h + 1) * K)
                dcol = dec_sb[:, h:h + 1]
                nc.vector.tensor_scalar(dd2[:, sl], nyb, dcol, None, op0=ADD)
                nc.vector.reciprocal_approx_fast(rec2[:, sl], dd2[:, sl])
                nc.vector.tensor_tensor(au2[:, sl], rec2[:, sl], wb, op=MULT)
                nc.vector.tensor_reduce(
                    s2[:, h:h + 1], au2[:, sl], axis=X_AX, op=ADD
                )
            nc.vector.reciprocal_approx_fast(srec2[:], s2[:])
            nc.vector.tensor_tensor(vsc2[:], v2, srec2[:], op=MULT)   # V/S
            nc.vector.tensor_tensor(sv2[:], s2[:], vinv2, op=MULT)    # S/V
            for h in range(2):
                sl = slice(h * K, (h + 1) * K)
                nc.vector.tensor_scalar(
                    t5[:, sl], au2[:, sl], vsc2[:, h:h + 1], None, op0=MULT
                )
            # g_ps[0, k] = sum_q l_k(dec_q) V_q  (both halves accumulated)
            psA = psp.tile([128, 512], F32, tag="psA")
            g_ps = psA[0:1, 0:K]
            nc.tensor.matmul(g_ps, onescol_r, t5[:, 0:K], start=True, stop=False)
            nc.tensor.matmul(g_ps, onescol_r, t5[:, K:2 * K], start=False, stop=True)
            gsb = cp.tile([1, K], F32, tag="gsb")   # g = cw * (row above)
            nc.vector.tensor_tensor(gsb[:], g_ps, cwrow_r, op=MULT)
            gbc_ps = psA[:, 16:16 + K]
            nc.tensor.matmul(gbc_ps, onesrow_f, gsb[:], start=True, stop=True)
            gbc = cp.tile([128, K], F32, tag="gbcsb")
            nc.vector.tensor_copy(gbc[:], gbc_ps)

            # tail-prep tiles (chains emitted interleaved inside the loop)
            svrow = cp.tile([1, D], F32R, tag="svrow")
            svbc_ps = svbcp.tile([128, D], F32, tag="svbc")
            encsv = cp.tile([T, D], F32, tag="encsv")
            decbc_ps = psp.tile([K, 256], F32, tag="psB")
            ddk = cp.tile([K, D], F32, tag="ddk")
            reck = cp.tile([K, D], F32, tag="reck")
            auk = cp.tile([K, D], F32, tag="auk")
            gcol = cp.tile([K, 1], F32, tag="gcol")
            grec = cp.tile([K, 1], F32, tag="grec")
            gsel = cp.tile([K, 1], F32, tag="gsel")
            gsel2 = cp.tile([K, 1], F32, tag="gsel2")
            cwg = cp.tile([K, 1], F32, tag="cwg")

            def emit_auk():
                # k-partition au (rowsum matmul rhs); decbc via PE outer
                nc.tensor.matmul(
                    decbc_ps[:], onesrowK, drow[:], start=True, stop=True
                )
                nc.vector.tensor_scalar(ddk[:], decbc_ps[:], nycv, None, op0=ADD)
                nc.vector.reciprocal_approx_fast(reck[:], ddk[:])
                nc.vector.tensor_scalar(auk[:], reck[:], wcv, None, op0=MULT)

            def emit_cwg():
                # cwg: cw/g for DVE-accum nodes (Hg holds g*H), cw for
                # ACT-accum nodes (plain H); selected via constant masks.
                gT_ps = psA[0:K, 288:289]
                nc.tensor.transpose(gT_ps, gsb[:], ident[0:1, 0:1])
                nc.vector.tensor_copy(gcol[:], gT_ps)
                nc.vector.reciprocal_approx_fast(grec[:], gcol[:])
                nc.vector.tensor_scalar(gsel[:], grec[:], maskcv, None, op0=MULT)
                nc.vector.tensor_tensor(gsel2[:], gsel[:], imaskcv, op=ADD)
                nc.vector.tensor_scalar(cwg[:], gsel2[:], cwcv, None, op0=MULT)

            def emit_sv():
                # SV broadcast + encSV
                for h in range(2):
                    svT_ps = psA[0:1, 32 + h * 128:32 + (h + 1) * 128]
                    nc.tensor.transpose(svT_ps, sv2[:, h:h + 1], ident)
                    nc.vector.tensor_copy(
                        svrow[:, h * 128:(h + 1) * 128], svT_ps
                    )
                nc.tensor.matmul(
                    svbc_ps[:], onesrow_r, svrow[:], start=True, stop=True
                )
                nc.vector.tensor_tensor(
                    encsv[:], enc_sb[:], svbc_ps[:], op=MULT
                )

            # ---- main loop: E_k -> GE_k (+H accum) -> PSUM colsum accumulate ----
            hg = cp.tile([128, K], F32, tag="hg")
            cacc_ps = caccp.tile([T, D], F32, tag="cacc")
            for k in range(K):
                e_t = ep.tile([T, D], F32, tag="e")
                if k < NDV:
                    nc.scalar.activation(
                        e_t[:], enc_sb[:], EXP, scale=float(Y_NODES[k]),
                        bias=zt[:, 0:1],
                    )
                else:
                    nc.scalar.activation(
                        e_t[:], enc_sb[:], EXP, scale=float(Y_NODES[k]),
                        bias=zt[:, 0:1], accum_out=hg[:, k:k + 1],
                    )
                ge_t = gep.tile([T, D], F32R, tag="ge")
                if k < NDV:
                    nc.vector.tensor_scalar(
                        ge_t[:], e_t[:], gbc[:, k:k + 1], 0.0, op0=MULT, op1=ADD,
                        accum_out=hg[:, k:k + 1],
                    )
                else:
                    nc.vector.tensor_scalar(
                        ge_t[:], e_t[:], gbc[:, k:k + 1], None, op0=MULT
                    )
                nc.tensor.matmul(
                    cacc_ps[:], identr, ge_t[:],
                    start=(k == 0), stop=(k == K - 1),
                )
                if k == 0:
                    emit_auk()
                elif k == 2:
                    emit_cwg()
                elif k == 4:
                    emit_sv()

            # ---- rowsum interp + combine ----
            hgT_ps = psA[0:K, 384:512]
            nc.tensor.transpose(hgT_ps, hg[:], ident)
            htw = cp.tile([K, 128], F32, tag="htw")
            nc.vector.tensor_scalar(htw[:], hgT_ps, cwg[:], None, op0=MULT)
            rsA = rsp.tile([128, 512], F32, tag="rsA")
            rs_ps = rsA[:, 0:256]
            nc.tensor.matmul(rs_ps, htw[:], auk[:], start=True, stop=True)
            rrec = cp.tile([T, D], F32, tag="rrec")
            nc.vector.reciprocal_approx_fast(rrec[:], rs_ps)
            c1 = cp.tile([T, D], F32, tag="c1")
            nc.vector.scalar_tensor_tensor(
                c1[:], cacc_ps[:], 1.0, rrec[:], op0=MULT, op1=MULT
            )
            contrib = cp.tile([T, D], F32R, tag="contrib")
            nc.vector.tensor_tensor(contrib[:], c1[:], encsv[:], op=MULT)
            fin_ps = rsA[0:1, 256:512]
            nc.tensor.matmul(fin_ps, onescol_r, contrib[:], start=True, stop=True)
            out_sb = cp.tile([1, D], F32, tag="outsb")
            nc.vector.tensor_copy(out_sb[:], fin_ps)
            nc.sync.dma_start(out, out_sb[:])
    nc.compile()
    return nc


_NC_CACHE = None


def _get_nc():
    global _NC_CACHE
    if _NC_CACHE is None:
        _NC_CACHE = build_nc()
    return _NC_CACHE


def make_in_maps(dec_t: np.ndarray, enc_out: np.ndarray):
    bf = _blob_f()
    br = _blob_r()
    in_maps = []
    for b in range(B):
        dec2 = np.stack([dec_t[b, :128], dec_t[b, 128:]], axis=1)
        in_maps.append(
            {
                "enc": np.ascontiguousarray(enc_out[b]).astype(np.float32),
                "decq": np.ascontiguousarray(dec2).astype(np.float32),
                "decrow": np.ascontiguousarray(
                    dec_t[b][None, :]
                ).astype(np.float32),
                "blob_f": bf,
                "blob_r": br,
            }
        )
    return in_maps


def run(dec_t: np.ndarray, enc_out: np.ndarray, **kwargs):
    """Run on all 8 cores; returns ([B, D] output, BassKernelResults)."""
    nc = _get_nc()
    res = run_bass_kernel_spmd(
        nc, make_in_maps(dec_t, enc_out), core_ids=list(range(NCORES)), **kwargs
    )
    out = np.stack([np.asarray(r["out"]).reshape(D) for r in res.results], axis=0)
    return out.astype(np.float32), res


def kernel(dec_t: np.ndarray, enc_out: np.ndarray) -> np.ndarray:
    dec_t = np.asarray(dec_t, dtype=np.float32)
    enc_out = np.asarray(enc_out, dtype=np.float32)
    out, _ = run(dec_t, enc_out)
    return out
